# revision 53
# baseline (speedup 1.0000x reference)
"""Trainium2 Bass kernel for nn_Node_Transformation.

Reference semantics, for row n:
    out[n] = x[n] @ W.T + b            if node_type[n] == item_id
             emb_weight[node_type[n]]  otherwise

Only ~1/8 of rows take the linear path; every other row is one of 7
constant 128-float vectors. The host-side sharding step therefore groups
each core's rows by node_type (selected rows first, then one contiguous
run per other type, each padded to a 128-row tile boundary). The device
kernel then:
  * reads ONLY the selected rows of x (pre-transposed to [256, S], cast
    to bf16), computes lin = x_sel @ W.T via PE-array matmuls, adds the
    bias in fp32 while moving PSUM->SBUF, and writes it to its own
    output tensor;
  * writes each constant run into a per-group output tensor by
    broadcast-source DMAs (stride-0 fan-out of a [128,1,128] replicated
    tile), groups statically spread over the gpsimd/sync/scalar DMA
    queues. Separate output tensors keep the write streams free of
    false write-write dependencies so all queues run concurrently.
The host scatters device rows back to their original positions.

HBM traffic per core: ~4.3 MB read + ~32.3 MB write -> memory-roofline
~103 us at 358 GB/s (vs ~96 MB and ~270 us for the dense formulation).
"""

import os
import numpy as np
import ml_dtypes

import concourse.bass as bass
import concourse.bacc as bacc
import concourse.mybir as mybir
from concourse.tile import TileContext
from concourse.bass_utils import run_bass_kernel_spmd

# ---- problem constants (hardcoded per contest contract) ----
N = 500000
IN_CH = 256
HID = 128
NUM_T = 8
NCORES = 8
P = 128
NSH = N // NCORES          # 62500 rows per core
KT = 4                     # 128-row tiles per PSUM accumulation group
KW = 16                    # tiles per write chunk (2048 rows, 1 MB)
KC = 8                     # real copies in a const source tile (2 KB segs)

_CACHE = {}


def _ensure_axon_profile_hook():
    """bass_utils' trace path imports antenv.axon_hooks, which this image
    lacks. Register an equivalent module backed by the axon PJRT .so so
    trace=True (or BASS_TRACE=1) works instead of crashing."""
    try:
        import antenv.axon_hooks  # noqa: F401
        return
    except ImportError:
        pass
    import sys
    import types

    hook = None
    try:
        from trn_agent_boot.trn_boot import _ntff_profile_via_ctypes

        hook = _ntff_profile_via_ctypes("/opt/axon/libaxon_pjrt.so")
    except Exception:
        hook = None
    mod = types.ModuleType("antenv.axon_hooks")
    mod.get_axon_ntff_profile_hook = lambda: hook
    mod.set_axon_ntff_profile_hook = lambda h: None
    sys.modules["antenv.axon_hooks"] = mod
    try:
        import antenv

        antenv.axon_hooks = mod
    except ImportError:
        pass


def _build(S: int, consts: tuple) -> bass.Bass:
    """S: selected-row region size (rows, multiple of KT*128).
    consts: tuple of per-group padded row counts (each a multiple of 128),
    one per non-selected node type, each written to its own output."""
    nc = bacc.Bacc("TRN2")
    f32 = mybir.dt.float32
    bf16 = mybir.dt.bfloat16
    ngroups = len(consts)

    xt_d = nc.dram_tensor("xt", [IN_CH, max(S, 1)], bf16, kind="ExternalInput")
    wt_d = nc.dram_tensor("wt", [IN_CH, HID], bf16, kind="ExternalInput")
    # single row holding every const group's 128-vector, in bf16 (DMA
    # never converts dtypes); the fp32 bias is separate for the exact
    # PSUM add.
    cbr_d = nc.dram_tensor("cbr", [1, max(ngroups, 1) * HID], bf16,
                           kind="ExternalInput")
    bias_d = nc.dram_tensor("bias", [1, HID], f32, kind="ExternalInput")
    # Outputs are stored bf16 (host upcasts): halves the dominant HBM
    # write traffic; adds ~0.4% relative error against a 2e-2 gate.
    outl_d = nc.dram_tensor("outl", [max(S, 1), HID], bf16,
                            kind="ExternalOutput")
    outc_d = [
        nc.dram_tensor(f"outc{t}", [consts[t], HID], bf16,
                       kind="ExternalOutput")
        for t in range(ngroups)
    ]

    # Partition-major view: DRAM row p*J + j holds partition p, slot j.
    # Each partition's slots are CONTIGUOUS in DRAM, so DMA packets are
    # J*512-byte segments instead of 512-byte ones (the DMA engines are
    # packet-rate limited, so long segments are what buys bandwidth).
    # The host undoes this layout with a cheap reshape/transpose.
    def pmajor(ten, nrows):
        return ten[0:nrows, :].rearrange("(p j) h -> p j h", p=P)

    with TileContext(nc) as tc:
        with (
            tc.tile_pool(name="singles", bufs=1) as singles,
            tc.tile_pool(name="xp", bufs=4) as xpool,
            tc.tile_pool(name="op", bufs=4) as opool,
            tc.tile_pool(name="ps", bufs=5, space="PSUM") as pspool,
        ):
            # Const tiles hold KC REAL copies per group so write-time DMA
            # packets get KC*256-byte contiguous source segments (packet
            # rate is what limits the DMA engines); only the outer repeat
            # dim is stride-0 at write time. One tiny partition-broadcast
            # fill (the program's first DMA) lands every group's vector;
            # four vector-engine copies (no DMA-engine cost) widen ALL
            # groups at once to KC copies each.
            ng = max(ngroups, 1)
            ct0 = singles.tile([P, ng, HID], bf16)
            nc.sync.dma_start(
                out=ct0[:],
                in_=cbr_d[:].rearrange("o (t h) -> o t h", h=HID)
                .to_broadcast([P, ng, HID]),
            )
            ct_all = singles.tile([P, ng, KC, HID], bf16)
            nc.vector.tensor_copy(ct_all[:, :, 0, :], ct0[:])
            w = 1
            while w < KC:
                nc.vector.tensor_copy(ct_all[:, :, w : 2 * w, :],
                                      ct_all[:, :, 0:w, :])
                w *= 2
            const_s = [ct_all[:, t, :, :] for t in range(ngroups)]
            wt_s = singles.tile([P, 2, HID], bf16)
            nc.sync.dma_start(
                out=wt_s[:], in_=wt_d[:].rearrange("(two c) h -> c two h", two=2)
            )
            bias_rep = singles.tile([P, 1, HID], f32)
            nc.sync.dma_start(
                out=bias_rep[:],
                in_=bias_d[:].rearrange("o (k h) -> o k h", k=1)
                .to_broadcast([P, 1, HID]),
            )

            # Linear region: S rows in super-groups of KW tiles (one write
            # chunk), each made of KT-tile PSUM accumulation groups.
            stiles = S // P
            for g in range(0, stiles, KW):
                w = min(KW, stiles - g)
                c0 = g * P
                xt0 = xpool.tile([P, KW, P], bf16, tag="x0")
                xt1 = xpool.tile([P, KW, P], bf16, tag="x1")
                nc.sync.dma_start(
                    out=xt0[:, 0:w, :],
                    in_=xt_d[0:P, c0 : c0 + w * P].rearrange(
                        "c (k p) -> c k p", k=w),
                )
                nc.sync.dma_start(
                    out=xt1[:, 0:w, :],
                    in_=xt_d[P : 2 * P, c0 : c0 + w * P].rearrange(
                        "c (k p) -> c k p", k=w),
                )
                o_t = opool.tile([P, KW, HID], bf16, tag="o")
                for q in range(0, w, KT):
                    ps = pspool.tile([P, KT, HID], f32, tag="ps")
                    for k in range(KT):
                        nc.tensor.matmul(out=ps[:, k, :],
                                         lhsT=xt0[:, q + k, :],
                                         rhs=wt_s[:, 0, :],
                                         start=True, stop=False)
                        nc.tensor.matmul(out=ps[:, k, :],
                                         lhsT=xt1[:, q + k, :],
                                         rhs=wt_s[:, 1, :],
                                         start=False, stop=True)
                    # PSUM -> SBUF move fused with the fp32 bias add.
                    nc.vector.tensor_tensor(
                        out=o_t[:, q : q + KT, :], in0=ps[:],
                        in1=bias_rep[:].to_broadcast([P, KT, HID]),
                        op=mybir.AluOpType.add)
                # Tile g from partition p lands at DRAM row p*stiles + g:
                # contiguous per-partition segments of w*512 bytes.
                nc.scalar.dma_start(
                    out=pmajor(outl_d, S)[:, g : g + w, :], in_=o_t[:, 0:w, :]
                )

            # Constant regions: broadcast-source writes per group (all
            # rows identical, so the partition-major layout is free).
            # Bulk chunks fan the KC-wide tile out m times; a tail DMA
            # covers the remaining j-slots. Scalar keeps only lin writes;
            # sync's reads are all done early, then it helps with consts.
            queues = [nc.gpsimd, nc.scalar, nc.sync]
            for t in range(ngroups):
                q = queues[t % len(queues)]
                jrows = consts[t] // P
                m, jt = divmod(jrows, KC)
                if m:
                    q.dma_start(
                        out=outc_d[t][0 : m * KC * P, :].rearrange(
                            "(p m k) h -> p m k h", p=P, k=KC),
                        in_=const_s[t][:].rearrange("p (m k) h -> p m k h", m=1)
                        .to_broadcast([P, m, KC, HID]),
                    )
                if jt:
                    q.dma_start(
                        out=outc_d[t][m * KC * P : jrows * P, :].rearrange(
                            "(p k) h -> p k h", p=P),
                        in_=const_s[t][:, 0:jt, :],
                    )
    nc.compile()
    return nc


def _round_up(v, m):
    return (v + m - 1) // m * m


def _prepare(inputs):
    x = np.ascontiguousarray(np.asarray(inputs["x"], dtype=np.float32))
    nt = np.asarray(inputs["node_type"]).astype(np.int64).ravel()
    item = int(np.asarray(inputs["item_id"]))
    emb = np.asarray(inputs["emb_weight"], dtype=np.float32)
    b = np.asarray(inputs["b"], dtype=np.float32)
    W = np.asarray(inputs["W"], dtype=np.float32)
    wt = np.ascontiguousarray(W.T.astype(ml_dtypes.bfloat16))  # [IN_CH, HID]

    const_types = [t for t in range(NUM_T) if t != item]

    sel_idx, grp_idx = [], []
    for c in range(NCORES):
        nt_c = nt[c * NSH : (c + 1) * NSH]
        sel_idx.append(np.flatnonzero(nt_c == item))
        grp_idx.append([np.flatnonzero(nt_c == t) for t in const_types])

    # 2048-row (KC*P) alignment keeps every partition-major DRAM segment
    # 4 KB-aligned, at the cost of ~3% filler rows.
    S = _round_up(max(len(s) for s in sel_idx), KC * P)
    consts = tuple(
        _round_up(max(len(grp_idx[c][g]) for c in range(NCORES)), KC * P)
        for g in range(len(const_types))
    )

    # One row: every const group's vector, in bf16; the bias stays fp32.
    ng = max(len(const_types), 1)
    cbr = np.zeros((1, ng * HID), ml_dtypes.bfloat16)
    for g, t in enumerate(const_types):
        cbr[0, g * HID : (g + 1) * HID] = emb[t].astype(ml_dtypes.bfloat16)
    bias = np.ascontiguousarray(b.reshape(1, HID), dtype=np.float32)

    in_maps = []
    for c in range(NCORES):
        xt = np.zeros((IN_CH, max(S, 1)), ml_dtypes.bfloat16)
        si = sel_idx[c]
        if len(si):
            xt[:, : len(si)] = x[c * NSH + si].T.astype(ml_dtypes.bfloat16)
        in_maps.append({"xt": xt, "wt": wt, "cbr": cbr, "bias": bias})
    return S, consts, sel_idx, grp_idx, in_maps


def _run(inputs, trace=False):
    _ensure_axon_profile_hook()
    S, consts, sel_idx, grp_idx, in_maps = _prepare(inputs)
    key = (S, consts)
    if key not in _CACHE:
        _CACHE[key] = _build(S, consts)
    nc = _CACHE[key]
    res = run_bass_kernel_spmd(nc, in_maps, core_ids=list(range(NCORES)),
                               trace=trace)
    out = np.empty((N, HID), np.float32)
    for c in range(NCORES):
        r = res.results[c]
        out_c = out[c * NSH : (c + 1) * NSH]
        si = sel_idx[c]
        if len(si):
            # Undo the device's partition-major lin layout: DRAM row
            # p*(S//128)+g holds logical selected row g*128+p.
            lin = r["outl"].astype(np.float32).reshape(P, S // P, HID)
            out_c[si] = lin.transpose(1, 0, 2).reshape(S, HID)[: len(si)]
        for g, gi in enumerate(grp_idx[c]):
            if len(gi):
                out_c[gi] = r[f"outc{g}"][: len(gi)].astype(np.float32)
    return out, res


def kernel(**inputs) -> np.ndarray:
    out, _ = _run(inputs, trace=bool(os.environ.get("KERNEL_TRACE")))
    return out


# revision 55
# speedup vs baseline: 1.1964x; 1.1964x over previous
"""Trainium2 Bass kernel for nn_Node_Transformation.

Reference semantics, for row n:
    out[n] = x[n] @ W.T + b            if node_type[n] == item_id
             emb_weight[node_type[n]]  otherwise

Only ~1/8 of rows take the linear path; every other row is one of 7
constant 128-float vectors. The host-side sharding step therefore groups
each core's rows by node_type (selected rows first, then one contiguous
run per other type, each padded to a 128-row tile boundary). The device
kernel then:
  * reads ONLY the selected rows of x (pre-transposed to [256, S], cast
    to bf16), computes lin = x_sel @ W.T via PE-array matmuls, adds the
    bias in fp32 while moving PSUM->SBUF, and writes it to its own
    output tensor;
  * writes each constant run into a per-group output tensor by
    broadcast-source DMAs that fan a [128, KC, 128] replicated tile out
    with a stride-0 outer repeat dim, groups statically spread over the
    gpsimd/scalar/sync DMA queues. Separate output tensors keep the
    write streams free of false write-write dependencies.
The host scatters device rows back to their original positions.

Hard-won tuning facts baked into the structure below:
  * The 16 per-core DMA engines are packet-rate limited (~16-25 GB/s
    each); aggregate ~390 GB/s needs >=2 KB contiguous segments on BOTH
    ends of every large transfer, hence the partition-major DRAM layout
    ("(p j) h") and the KC real copies in the const source tiles.
  * Outputs are stored bf16 and upcast on the host: halves the dominant
    write traffic for ~0.4% extra relative error (gate is 2e-2).
  * Setup must be a single early DMA plus vector-engine-only work: any
    multi-step setup chain touching a DMA queue or PSUM gets inverted
    behind the bulk write stream by the Tile scheduler's static order.
Per-core DMA bytes: ~4.2 MB x reads + ~16.4 MB writes + ~0.5 MB misc
=> ~54 us at the measured engine roofline; measured ~67-70 us total
(vs 728 us for the staged dense-formulation baseline).
"""

import os
import numpy as np
import ml_dtypes

import concourse.bass as bass
import concourse.bacc as bacc
import concourse.mybir as mybir
from concourse.tile import TileContext
from concourse.bass_utils import run_bass_kernel_spmd

# ---- problem constants (hardcoded per contest contract) ----
N = 500000
IN_CH = 256
HID = 128
NUM_T = 8
NCORES = 8
P = 128
NSH = N // NCORES          # 62500 rows per core
KT = 4                     # 128-row tiles per PSUM accumulation group
KW = 16                    # tiles per write chunk (2048 rows, 1 MB)
KC = 8                     # real copies in a const source tile (2 KB segs)

_CACHE = {}


def _ensure_axon_profile_hook():
    """bass_utils' trace path imports antenv.axon_hooks, which this image
    lacks. Register an equivalent module backed by the axon PJRT .so so
    trace=True (or BASS_TRACE=1) works instead of crashing."""
    try:
        import antenv.axon_hooks  # noqa: F401
        return
    except ImportError:
        pass
    import sys
    import types

    hook = None
    try:
        from trn_agent_boot.trn_boot import _ntff_profile_via_ctypes

        hook = _ntff_profile_via_ctypes("/opt/axon/libaxon_pjrt.so")
    except Exception:
        hook = None
    mod = types.ModuleType("antenv.axon_hooks")
    mod.get_axon_ntff_profile_hook = lambda: hook
    mod.set_axon_ntff_profile_hook = lambda h: None
    sys.modules["antenv.axon_hooks"] = mod
    try:
        import antenv

        antenv.axon_hooks = mod
    except ImportError:
        pass


def _build(S: int, consts: tuple) -> bass.Bass:
    """S: selected-row region size (rows, multiple of KT*128).
    consts: tuple of per-group padded row counts (each a multiple of 128),
    one per non-selected node type, each written to its own output."""
    nc = bacc.Bacc("TRN2")
    f32 = mybir.dt.float32
    bf16 = mybir.dt.bfloat16
    ngroups = len(consts)

    xt_d = nc.dram_tensor("xt", [IN_CH, max(S, 1)], bf16, kind="ExternalInput")
    wt_d = nc.dram_tensor("wt", [IN_CH, HID], bf16, kind="ExternalInput")
    # single row holding every const group's 128-vector, in bf16 (DMA
    # never converts dtypes); the fp32 bias is separate for the exact
    # PSUM add.
    cbr_d = nc.dram_tensor("cbr", [1, max(ngroups, 1) * HID], bf16,
                           kind="ExternalInput")
    bias_d = nc.dram_tensor("bias", [1, HID], f32, kind="ExternalInput")
    # Outputs are stored bf16 (host upcasts): halves the dominant HBM
    # write traffic; adds ~0.4% relative error against a 2e-2 gate.
    outl_d = nc.dram_tensor("outl", [max(S, 1), HID], bf16,
                            kind="ExternalOutput")
    outc_d = [
        nc.dram_tensor(f"outc{t}", [consts[t], HID], bf16,
                       kind="ExternalOutput")
        for t in range(ngroups)
    ]

    # Partition-major view: DRAM row p*J + j holds partition p, slot j.
    # Each partition's slots are CONTIGUOUS in DRAM, so DMA packets are
    # J*512-byte segments instead of 512-byte ones (the DMA engines are
    # packet-rate limited, so long segments are what buys bandwidth).
    # The host undoes this layout with a cheap reshape/transpose.
    def pmajor(ten, nrows):
        return ten[0:nrows, :].rearrange("(p j) h -> p j h", p=P)

    with TileContext(nc) as tc:
        with (
            tc.tile_pool(name="singles", bufs=1) as singles,
            tc.tile_pool(name="xp", bufs=4) as xpool,
            tc.tile_pool(name="op", bufs=4) as opool,
            tc.tile_pool(name="ps", bufs=5, space="PSUM") as pspool,
        ):
            # Const tiles hold KC REAL copies per group so write-time DMA
            # packets get KC*256-byte contiguous source segments (packet
            # rate is what limits the DMA engines); only the outer repeat
            # dim is stride-0 at write time. One tiny partition-broadcast
            # fill (the program's first DMA) lands every group's vector;
            # four vector-engine copies (no DMA-engine cost) widen ALL
            # groups at once to KC copies each.
            ng = max(ngroups, 1)
            ct0 = singles.tile([P, ng, HID], bf16)
            nc.sync.dma_start(
                out=ct0[:],
                in_=cbr_d[:].rearrange("o (t h) -> o t h", h=HID)
                .to_broadcast([P, ng, HID]),
            )
            ct_all = singles.tile([P, ng, KC, HID], bf16)
            nc.vector.tensor_copy(ct_all[:, :, 0, :], ct0[:])
            w = 1
            while w < KC:
                nc.vector.tensor_copy(ct_all[:, :, w : 2 * w, :],
                                      ct_all[:, :, 0:w, :])
                w *= 2
            const_s = [ct_all[:, t, :, :] for t in range(ngroups)]
            wt_s = singles.tile([P, 2, HID], bf16)
            nc.sync.dma_start(
                out=wt_s[:], in_=wt_d[:].rearrange("(two c) h -> c two h", two=2)
            )
            bias_rep = singles.tile([P, 1, HID], f32)
            nc.sync.dma_start(
                out=bias_rep[:],
                in_=bias_d[:].rearrange("o (k h) -> o k h", k=1)
                .to_broadcast([P, 1, HID]),
            )

            # Linear region: S rows in super-groups of KW tiles (one write
            # chunk), each made of KT-tile PSUM accumulation groups.
            stiles = S // P
            for g in range(0, stiles, KW):
                w = min(KW, stiles - g)
                c0 = g * P
                xt0 = xpool.tile([P, KW, P], bf16, tag="x0")
                xt1 = xpool.tile([P, KW, P], bf16, tag="x1")
                nc.sync.dma_start(
                    out=xt0[:, 0:w, :],
                    in_=xt_d[0:P, c0 : c0 + w * P].rearrange(
                        "c (k p) -> c k p", k=w),
                )
                nc.sync.dma_start(
                    out=xt1[:, 0:w, :],
                    in_=xt_d[P : 2 * P, c0 : c0 + w * P].rearrange(
                        "c (k p) -> c k p", k=w),
                )
                o_t = opool.tile([P, KW, HID], bf16, tag="o")
                for q in range(0, w, KT):
                    ps = pspool.tile([P, KT, HID], f32, tag="ps")
                    for k in range(KT):
                        nc.tensor.matmul(out=ps[:, k, :],
                                         lhsT=xt0[:, q + k, :],
                                         rhs=wt_s[:, 0, :],
                                         start=True, stop=False)
                        nc.tensor.matmul(out=ps[:, k, :],
                                         lhsT=xt1[:, q + k, :],
                                         rhs=wt_s[:, 1, :],
                                         start=False, stop=True)
                    # PSUM -> SBUF move fused with the fp32 bias add.
                    nc.vector.tensor_tensor(
                        out=o_t[:, q : q + KT, :], in0=ps[:],
                        in1=bias_rep[:].to_broadcast([P, KT, HID]),
                        op=mybir.AluOpType.add)
                # Tile g from partition p lands at DRAM row p*stiles + g:
                # contiguous per-partition segments of w*512 bytes.
                nc.scalar.dma_start(
                    out=pmajor(outl_d, S)[:, g : g + w, :], in_=o_t[:, 0:w, :]
                )

            # Constant regions: broadcast-source writes per group (all
            # rows identical, so the partition-major layout is free).
            # Bulk chunks fan the KC-wide tile out m times; a tail DMA
            # covers the remaining j-slots. Scalar keeps only lin writes;
            # sync's reads are all done early, then it helps with consts.
            queues = [nc.gpsimd, nc.scalar, nc.sync]
            for t in range(ngroups):
                q = queues[t % len(queues)]
                jrows = consts[t] // P
                m, jt = divmod(jrows, KC)
                if m:
                    q.dma_start(
                        out=outc_d[t][0 : m * KC * P, :].rearrange(
                            "(p m k) h -> p m k h", p=P, k=KC),
                        in_=const_s[t][:].rearrange("p (m k) h -> p m k h", m=1)
                        .to_broadcast([P, m, KC, HID]),
                    )
                if jt:
                    q.dma_start(
                        out=outc_d[t][m * KC * P : jrows * P, :].rearrange(
                            "(p k) h -> p k h", p=P),
                        in_=const_s[t][:, 0:jt, :],
                    )
    nc.compile()
    return nc


def _round_up(v, m):
    return (v + m - 1) // m * m


def _prepare(inputs):
    x = np.ascontiguousarray(np.asarray(inputs["x"], dtype=np.float32))
    nt = np.asarray(inputs["node_type"]).astype(np.int64).ravel()
    item = int(np.asarray(inputs["item_id"]))
    emb = np.asarray(inputs["emb_weight"], dtype=np.float32)
    b = np.asarray(inputs["b"], dtype=np.float32)
    W = np.asarray(inputs["W"], dtype=np.float32)
    wt = np.ascontiguousarray(W.T.astype(ml_dtypes.bfloat16))  # [IN_CH, HID]

    const_types = [t for t in range(NUM_T) if t != item]

    sel_idx, grp_idx = [], []
    for c in range(NCORES):
        nt_c = nt[c * NSH : (c + 1) * NSH]
        sel_idx.append(np.flatnonzero(nt_c == item))
        grp_idx.append([np.flatnonzero(nt_c == t) for t in const_types])

    S = _round_up(max(len(s) for s in sel_idx), KT * P)
    consts = tuple(
        _round_up(max(len(grp_idx[c][g]) for c in range(NCORES)), P)
        for g in range(len(const_types))
    )

    # One row: every const group's vector, in bf16; the bias stays fp32.
    ng = max(len(const_types), 1)
    cbr = np.zeros((1, ng * HID), ml_dtypes.bfloat16)
    for g, t in enumerate(const_types):
        cbr[0, g * HID : (g + 1) * HID] = emb[t].astype(ml_dtypes.bfloat16)
    bias = np.ascontiguousarray(b.reshape(1, HID), dtype=np.float32)

    in_maps = []
    for c in range(NCORES):
        xt = np.zeros((IN_CH, max(S, 1)), ml_dtypes.bfloat16)
        si = sel_idx[c]
        if len(si):
            xt[:, : len(si)] = x[c * NSH + si].T.astype(ml_dtypes.bfloat16)
        in_maps.append({"xt": xt, "wt": wt, "cbr": cbr, "bias": bias})
    return S, consts, sel_idx, grp_idx, in_maps


def _run(inputs, trace=False):
    _ensure_axon_profile_hook()
    S, consts, sel_idx, grp_idx, in_maps = _prepare(inputs)
    key = (S, consts)
    if key not in _CACHE:
        _CACHE[key] = _build(S, consts)
    nc = _CACHE[key]
    res = run_bass_kernel_spmd(nc, in_maps, core_ids=list(range(NCORES)),
                               trace=trace)
    out = np.empty((N, HID), np.float32)
    for c in range(NCORES):
        r = res.results[c]
        out_c = out[c * NSH : (c + 1) * NSH]
        si = sel_idx[c]
        if len(si):
            # Undo the device's partition-major lin layout: DRAM row
            # p*(S//128)+g holds logical selected row g*128+p.
            lin = r["outl"].astype(np.float32).reshape(P, S // P, HID)
            out_c[si] = lin.transpose(1, 0, 2).reshape(S, HID)[: len(si)]
        for g, gi in enumerate(grp_idx[c]):
            if len(gi):
                out_c[gi] = r[f"outc{g}"][: len(gi)].astype(np.float32)
    return out, res


def kernel(**inputs) -> np.ndarray:
    out, _ = _run(inputs, trace=bool(os.environ.get("KERNEL_TRACE")))
    return out
